# revision 10
# baseline (speedup 1.0000x reference)
"""MoE-ALU (add with carry + xor over one-hot byte encodings) on 8 NeuronCores.

Semantics (validated against the jax reference bit-exactly): inputs a, b are
exact one-hot byte encodings [B, 4, 256] (little-endian bytes of 32-bit ints);
with SCALE=100 every softmax in the reference collapses to an exact one-hot, so

    out[0] = one_hot bytes of (a_int + b_int) mod 2^32
    out[1] = one_hot bytes of (a_int ^ b_int)

Device kernel (pure data parallel, batch sharded over 8 cores), raw Bass
(this toolchain's walrus encodes at most ONE sync wait per instruction, so
Tile-generated schedules don't compile; manual sems with standalone waits do).

v2 pipeline (HBM-roofline oriented: 32 MB in + 32 MB out per core at
~358 GB/s => ~190 us floor; v1 stalled DMA ~20 us in the tail because DVE
busy (185 us) matched DMA busy):

  per 128-row tile (a|b side by side in one [128, 2048] SBUF tile):
    decode  4x scalar_tensor_tensor f32 with accum_out -> a_lo a_hi b_lo b_hi
    add/xor int halves; carry folded via one (s_lo>=65536)+s_hi STT
    bytes   shift/mask -> 8 byte indices as f32
    encode  8x tensor_scalar is_equal over a bf16 iota[256] stream against the
            per-partition byte index (AP scalar) -> bf16 one-hots (16-bit
            dtype runs the DVE in a packed perf mode, ~2x over the v1
            f32 is_equal, and per-partition scalars avoid stride-0 streams)
    cast    ACT engine (own SBUF ports, otherwise idle) copies bf16 -> f32
            out tiles, then issues the output DMAs (HWDGE)

  engines: SyncE issues input DMAs (released as soon as a tile's decode is
  done, via s_dve thresholds), ScalarE(ACT) casts + issues output DMAs,
  VectorE computes. Rotating per-buffer-slot semaphores make DMA-queue
  completion order irrelevant.

  DVE ops overlap in the engine pipe and do NOT self-interlock (measured:
  removing sync gives stale reads), so every same-engine RAW step waits on a
  monotonically counted DVE semaphore; per-tile temporaries are
  parity-double-buffered. ACT likewise self-waits on s_cast between the cast
  copy and the dma_start that reads it.
"""
from contextlib import ExitStack

import numpy as np

import concourse.bass as bass
from concourse import mybir
from concourse.bass_utils import run_bass_kernel_spmd

F32 = mybir.dt.float32
I32 = mybir.dt.int32
BF16 = mybir.dt.bfloat16

P = 128
N_CORES = 8
B = 32768
B_LOC = B // N_CORES          # 4096 rows per core
ROW = 4 * 256                 # 1024 f32 per row per tensor
N_TILES = B_LOC // P          # 32

NBUF = 9                      # input buffer slots
OBUF = 8                      # f32 output buffer slots
BFBUF = 4                     # bf16 encode buffer slots

TABI_COLS = 256 + 8           # encode iota | shift pattern


def _dve_counts():
    """Static s_dve schedule: per-tile increments on the vector engine.

    Per tile: 4 decode STTs, 1 cast, add+xor, and+isge, shift, mask, idx cast
    = 12; tile 0 has one extra (the one-time i32->bf16 iota table copy).
    Returns (start[i], dec_done[i], total_after[i]).
    """
    start, dec_done, after = [], [], []
    n = 0
    for i in range(N_TILES):
        start.append(n)
        n += 4
        dec_done.append(n)
        n += 5                 # cast, add, xor, and, isge
        if i == 0:
            n += 1             # enc_bf table copy
        n += 3                 # shift, mask, idx->f32 cast
        after.append(n)
    return start, dec_done, after


def _build_nc() -> bass.Bass:
    nc = bass.Bass(trn_type="TRN2")
    a_d = nc.dram_tensor("a", [B_LOC, ROW], F32, kind="ExternalInput")
    b_d = nc.dram_tensor("b", [B_LOC, ROW], F32, kind="ExternalInput")
    tabf_d = nc.dram_tensor("tabf", [P, 512], F32, kind="ExternalInput")
    tabi_d = nc.dram_tensor("tabi", [P, TABI_COLS], I32, kind="ExternalInput")
    out_d = nc.dram_tensor("out", [2, B_LOC, ROW], F32, kind="ExternalOutput")

    _, dec_done, _ = _dve_counts()

    with ExitStack() as ctx:
        sb = lambda name, shape, dt: ctx.enter_context(
            nc.sbuf_tensor(name, shape, dt))
        tabf_t = sb("tabf_t", [P, 512], F32)
        tabi_t = sb("tabi_t", [P, TABI_COLS], I32)
        enc_bf = sb("enc_bf", [P, 256], BF16)
        ab_t = [sb(f"ab_t{k}", [P, 2 * ROW], F32) for k in range(NBUF)]
        obf_t = [sb(f"obf_t{k}", [P, 2 * ROW], BF16) for k in range(BFBUF)]
        out_t = [sb(f"out_t{k}", [P, 2 * ROW], F32) for k in range(OBUF)]
        dump = [[sb(f"dump{p}_{k}", [P, 512], F32) for k in range(4)]
                for p in range(2)]
        # parity-double-buffered per-tile temporaries
        t6 = [sb(f"t6_{p}", [P, 6], F32) for p in range(2)]
        iv = [sb(f"iv_{p}", [P, 6], I32) for p in range(2)]   # a16 b16 s16
        v4 = [sb(f"v4_{p}", [P, 4], I32) for p in range(2)]   # slo shi xlo xhi
        sh8 = [sb(f"sh8_{p}", [P, 8], I32) for p in range(2)]
        idx8 = [sb(f"idx8_{p}", [P, 8], I32) for p in range(2)]
        idx8f = [sb(f"idx8f_{p}", [P, 8], F32) for p in range(2)]

        dec = tabf_t[:]
        enci = tabi_t[:, 0:256]
        shifts = tabi_t[:, 256:264].rearrange("p (a two) -> p a two", two=2)

        s_tab = ctx.enter_context(nc.semaphore("s_tab"))
        s_tab2 = ctx.enter_context(nc.semaphore("s_tab2"))
        s_load = [ctx.enter_context(nc.semaphore(f"s_load{j}"))
                  for j in range(NBUF)]
        s_store = [ctx.enter_context(nc.semaphore(f"s_store{j}"))
                   for j in range(OBUF)]
        s_comp = ctx.enter_context(nc.semaphore("s_comp"))
        s_cast = ctx.enter_context(nc.semaphore("s_cast"))
        s_dve = ctx.enter_context(nc.semaphore("s_dve"))

        block = ctx.enter_context(nc.Block())

        @block.sync
        def _(sync: bass.BassEngine):
            for i in range(N_TILES):
                j = i % NBUF
                if i >= NBUF:
                    # slot reuse: tile i-NBUF's decode consumed the slot
                    sync.wait_ge(s_dve, dec_done[i - NBUF])
                r0 = i * P
                sync.dma_start(
                    out=ab_t[j][:, 0:ROW], in_=a_d[r0:r0 + P, :]
                ).then_inc(s_load[j], 16)
                sync.dma_start(
                    out=ab_t[j][:, ROW:2 * ROW], in_=b_d[r0:r0 + P, :]
                ).then_inc(s_load[j], 16)
                if i == 0:
                    # tables after tile-0 data so compute starts sooner
                    sync.dma_start(
                        out=tabf_t[:], in_=tabf_d[:]).then_inc(s_tab, 16)
                elif i == 1:
                    sync.dma_start(
                        out=tabi_t[:], in_=tabi_d[:]).then_inc(s_tab2, 16)

        @block.scalar
        def _(scalar: bass.BassEngine):
            for i in range(N_TILES):
                jo = i % OBUF
                ib = i % BFBUF
                r0 = i * P
                scalar.wait_ge(s_comp, 2 * i + 1)
                if i >= OBUF:
                    scalar.wait_ge(s_store[jo], 32 * (i // OBUF))
                scalar.activation(
                    out=out_t[jo][:, 0:ROW], in_=obf_t[ib][:, 0:ROW],
                    func=mybir.ActivationFunctionType.Copy,
                ).then_inc(s_cast, 1)
                scalar.wait_ge(s_cast, 2 * i + 1)   # copy retired before DMA
                scalar.dma_start(
                    out=out_d[0, r0:r0 + P, :], in_=out_t[jo][:, 0:ROW]
                ).then_inc(s_store[jo], 16)
                scalar.wait_ge(s_comp, 2 * i + 2)
                scalar.activation(
                    out=out_t[jo][:, ROW:2 * ROW], in_=obf_t[ib][:, ROW:2 * ROW],
                    func=mybir.ActivationFunctionType.Copy,
                ).then_inc(s_cast, 1)
                scalar.wait_ge(s_cast, 2 * i + 2)
                scalar.dma_start(
                    out=out_d[1, r0:r0 + P, :], in_=out_t[jo][:, ROW:2 * ROW]
                ).then_inc(s_store[jo], 16)

        @block.vector
        def _(vector: bass.BassEngine):
            n = 0  # statically tracked s_dve count

            vector.wait_ge(s_tab, 16)   # dec table (loaded first)
            for i in range(N_TILES):
                j = i % NBUF
                ib = i % BFBUF
                pr = i % 2
                if i >= 2:
                    # tile i-2 (same parity) fully retired, incl. its encode,
                    # before its temporaries are reused
                    vector.wait_ge(s_comp, 2 * (i - 1))
                if i >= BFBUF:
                    # ACT consumed the bf16 slot of tile i-BFBUF
                    vector.wait_ge(s_cast, 2 * (i - BFBUF) + 2)
                vector.wait_ge(s_load[j], 32 * (i // NBUF + 1))

                # decode: a_lo a_hi b_lo b_hi as f32 accumulators
                for k in range(4):
                    vector.scalar_tensor_tensor(
                        out=dump[pr][k][:],
                        in0=ab_t[j][:, 512 * k:512 * k + 512],
                        scalar=1.0,
                        in1=dec,
                        op0=mybir.AluOpType.mult,
                        op1=mybir.AluOpType.mult,
                        accum_out=t6[pr][:, k:k + 1],
                    ).then_inc(s_dve, 1)
                n += 4
                vector.wait_ge(s_dve, n)
                # int cast of the four halves
                vector.tensor_copy(iv[pr][:, 0:4], t6[pr][:, 0:4]).then_inc(
                    s_dve, 1)
                n += 1
                vector.wait_ge(s_dve, n)
                # s16 halves sum and xor halves
                vector.tensor_tensor(
                    out=iv[pr][:, 4:6], in0=iv[pr][:, 0:2],
                    in1=iv[pr][:, 2:4],
                    op=mybir.AluOpType.add).then_inc(s_dve, 1)
                vector.tensor_tensor(
                    out=v4[pr][:, 2:4], in0=iv[pr][:, 0:2],
                    in1=iv[pr][:, 2:4],
                    op=mybir.AluOpType.bitwise_xor).then_inc(s_dve, 1)
                n += 2
                vector.wait_ge(s_dve, n)
                # carry lo->hi: s_lo' = s_lo & 65535 ; s_hi' = (s_lo>=2^16)+s_hi
                vector.tensor_scalar(
                    out=v4[pr][:, 0:1], in0=iv[pr][:, 4:5], scalar1=65535,
                    scalar2=None,
                    op0=mybir.AluOpType.bitwise_and).then_inc(s_dve, 1)
                vector.scalar_tensor_tensor(
                    out=v4[pr][:, 1:2], in0=iv[pr][:, 4:5], scalar=65536,
                    in1=iv[pr][:, 5:6],
                    op0=mybir.AluOpType.is_ge,
                    op1=mybir.AluOpType.add).then_inc(s_dve, 1)
                n += 2
                vector.wait_ge(s_dve, n)
                if i == 0:
                    vector.wait_ge(s_tab2, 16)  # shift/enc table ready
                    vector.tensor_copy(enc_bf[:], enci).then_inc(s_dve, 1)
                    n += 1
                vector.tensor_tensor(
                    out=sh8[pr][:],
                    in0=v4[pr][:, :, None].to_broadcast((P, 4, 2)),
                    in1=shifts,
                    op=mybir.AluOpType.logical_shift_right).then_inc(s_dve, 1)
                n += 1
                vector.wait_ge(s_dve, n)
                vector.tensor_scalar(
                    out=idx8[pr][:], in0=sh8[pr][:], scalar1=255,
                    scalar2=None,
                    op0=mybir.AluOpType.bitwise_and).then_inc(s_dve, 1)
                n += 1
                vector.wait_ge(s_dve, n)
                # byte indices as f32 for the is_equal AP scalar
                vector.tensor_copy(idx8f[pr][:], idx8[pr][:]).then_inc(
                    s_dve, 1)
                n += 1
                vector.wait_ge(s_dve, n)
                # encode: 8 per-byte one-hots in bf16; the iota stream is
                # bf16-packed, the byte index rides as a per-partition scalar
                for e in range(8):
                    ins = vector.tensor_scalar(
                        out=obf_t[ib][:, 256 * e:256 * e + 256],
                        in0=enc_bf[:],
                        scalar1=idx8f[pr][:, e:e + 1],
                        scalar2=None,
                        op0=mybir.AluOpType.is_equal)
                    if e == 3 or e == 7:
                        ins.then_inc(s_comp, 1)

    return nc


def _make_tables():
    dec = np.concatenate([np.arange(256), np.arange(256) * 256]).astype(np.float32)
    tabf = np.tile(dec[None, :], (P, 1))
    enc = np.arange(256, dtype=np.int64)
    shifts = np.array([0, 8] * 4, np.int64)
    tabi = np.tile(np.concatenate([enc, shifts]).astype(np.int32)[None, :],
                   (P, 1))
    return tabf, tabi


_NC_CACHE = {}


def _get_nc(variant: str = "main"):
    if variant not in _NC_CACHE:
        _NC_CACHE[variant] = _build_nc()
    return _NC_CACHE[variant]


def _run(a: np.ndarray, b: np.ndarray, **spmd_kwargs):
    assert a.shape == (B, 4, 256) and b.shape == (B, 4, 256)
    a2 = np.ascontiguousarray(a, dtype=np.float32).reshape(B, ROW)
    b2 = np.ascontiguousarray(b, dtype=np.float32).reshape(B, ROW)
    tabf, tabi = _make_tables()
    in_maps = [
        {
            "a": a2[i * B_LOC:(i + 1) * B_LOC],
            "b": b2[i * B_LOC:(i + 1) * B_LOC],
            "tabf": tabf,
            "tabi": tabi,
        }
        for i in range(N_CORES)
    ]
    nc = _get_nc()
    kr = run_bass_kernel_spmd(nc, in_maps, list(range(N_CORES)), **spmd_kwargs)
    shards = [kr.results[i]["out"] for i in range(N_CORES)]
    out = np.concatenate(shards, axis=1).reshape(2, B, 4, 256)
    return out, kr


def kernel(a: np.ndarray, b: np.ndarray) -> np.ndarray:
    out, _ = _run(a, b)
    return out


# revision 11
# speedup vs baseline: 1.1392x; 1.1392x over previous
"""MoE-ALU (add with carry + xor over one-hot byte encodings) on 8 NeuronCores.

Semantics (validated against the jax reference bit-exactly): inputs a, b are
exact one-hot byte encodings [B, 4, 256] (little-endian bytes of 32-bit ints);
with SCALE=100 every softmax in the reference collapses to an exact one-hot, so

    out[0] = one_hot bytes of (a_int + b_int) mod 2^32
    out[1] = one_hot bytes of (a_int ^ b_int)

Device kernel (pure data parallel, batch sharded over 8 cores), raw Bass
(this toolchain's walrus encodes at most ONE sync wait per instruction, so
Tile-generated schedules don't compile; manual sems with standalone waits do).

HBM roofline: 32 MB in + 32 MB out per core at ~358 GB/s -> ~190 us floor.
v1 (227.6 us) stalled DMA ~20 us in the tail because DVE busy (185 us)
matched DMA busy (200 us). v3 trims DVE work and stalls so DMA paces:

  per 128-row tile (a|b side by side in one [128, 2048] SBUF tile):
    decode  4x scalar_tensor_tensor with accum_out: multiply one 512-col
            segment by the [0..255 | 0,256,...,65280] pattern and reduce in
            one op -> a_lo a_hi b_lo b_hi (16-bit halves, f32-exact)
    add     int halves add straight into v4[0:2]; carry folded into the high
            half only, in place, via one (s_lo>=65536)+s_hi STT (the raw
            s_lo's bit 16 never survives the later >>0/>>8 then &255, so the
            low half needs no mask -- one op fewer than v1)
    xor     int32 xor of the halves
    bytes   int32 shift/mask -> 8 byte indices
    encode  two is_equal [128, 4, 256] of the int iota table against the
            stride-0-broadcast indices, writing f32 one-hots directly

  engines: SyncE issues input DMAs (tile-0 data before the tables so compute
  starts sooner; ab slots are released as soon as a tile's decode is done,
  via static s_dve thresholds), ScalarE issues output DMAs, VectorE computes.
  Rotating per-buffer-slot semaphores make DMA-queue completion order
  irrelevant (slot reuse is gated by the compute semaphore).

  DVE ops overlap in the engine pipe and do NOT self-interlock (measured:
  removing sync gives stale reads), so every same-engine RAW step waits on a
  monotonically counted DVE semaphore; per-tile temporaries are
  parity-double-buffered so consecutive tiles can overlap, with cross-parity
  reuse gated by the compute semaphore of tile i-1.
"""
from contextlib import ExitStack

import numpy as np

import concourse.bass as bass
from concourse import mybir
from concourse.bass_utils import run_bass_kernel_spmd

F32 = mybir.dt.float32
I32 = mybir.dt.int32

P = 128
N_CORES = 8
B = 32768
B_LOC = B // N_CORES          # 4096 rows per core
ROW = 4 * 256                 # 1024 f32 per row per tensor
N_TILES = B_LOC // P          # 32

NBUF = 9                      # input buffer slots
OBUF = 8                      # output buffer slots

TABI_COLS = 2048 + 8          # encode iota x8 | shift pattern

# static s_dve schedule: per-tile vector-engine increments
# (4 decode STTs, cast, add, xor, isge, shift, mask)
DVE_PER_TILE = 10
DVE_DEC_DONE = [DVE_PER_TILE * i + 4 for i in range(N_TILES)]


def _build_nc() -> bass.Bass:
    nc = bass.Bass(trn_type="TRN2")
    a_d = nc.dram_tensor("a", [B_LOC, ROW], F32, kind="ExternalInput")
    b_d = nc.dram_tensor("b", [B_LOC, ROW], F32, kind="ExternalInput")
    tabf_d = nc.dram_tensor("tabf", [P, 512], F32, kind="ExternalInput")
    tabi_d = nc.dram_tensor("tabi", [P, TABI_COLS], I32, kind="ExternalInput")
    out_d = nc.dram_tensor("out", [2, B_LOC, ROW], F32, kind="ExternalOutput")

    with ExitStack() as ctx:
        sb = lambda name, shape, dt: ctx.enter_context(
            nc.sbuf_tensor(name, shape, dt))
        tabf_t = sb("tabf_t", [P, 512], F32)
        tabi_t = sb("tabi_t", [P, TABI_COLS], I32)
        ab_t = [sb(f"ab_t{k}", [P, 2 * ROW], F32) for k in range(NBUF)]
        out_t = [sb(f"out_t{k}", [P, 2 * ROW], F32) for k in range(OBUF)]
        dump = [[sb(f"dump{p}_{k}", [P, 512], F32) for k in range(4)]
                for p in range(2)]
        # parity-double-buffered per-tile temporaries
        t6 = [sb(f"t6_{p}", [P, 4], F32) for p in range(2)]
        iv = [sb(f"iv_{p}", [P, 4], I32) for p in range(2)]   # a16 b16
        v4 = [sb(f"v4_{p}", [P, 4], I32) for p in range(2)]   # slo shi xlo xhi
        sh8 = [sb(f"sh8_{p}", [P, 8], I32) for p in range(2)]
        idx8 = [sb(f"idx8_{p}", [P, 8], I32) for p in range(2)]

        dec = tabf_t[:]
        enc = tabi_t[:, 0:2048].rearrange("p (e k) -> p e k", k=256)
        shifts = tabi_t[:, 2048:2056].rearrange("p (a two) -> p a two", two=2)

        s_tab = ctx.enter_context(nc.semaphore("s_tab"))
        s_tab2 = ctx.enter_context(nc.semaphore("s_tab2"))
        s_load = [ctx.enter_context(nc.semaphore(f"s_load{j}"))
                  for j in range(NBUF)]
        s_store = [ctx.enter_context(nc.semaphore(f"s_store{j}"))
                   for j in range(OBUF)]
        s_comp = ctx.enter_context(nc.semaphore("s_comp"))
        s_dve = ctx.enter_context(nc.semaphore("s_dve"))

        block = ctx.enter_context(nc.Block())

        @block.sync
        def _(sync: bass.BassEngine):
            for i in range(N_TILES):
                j = i % NBUF
                if i >= NBUF:
                    # slot reuse: tile i-NBUF's decode consumed the slot
                    sync.wait_ge(s_dve, DVE_DEC_DONE[i - NBUF])
                r0 = i * P
                sync.dma_start(
                    out=ab_t[j][:, 0:ROW], in_=a_d[r0:r0 + P, :]
                ).then_inc(s_load[j], 16)
                sync.dma_start(
                    out=ab_t[j][:, ROW:2 * ROW], in_=b_d[r0:r0 + P, :]
                ).then_inc(s_load[j], 16)
                if i == 0:
                    # tables after tile-0 data so compute starts sooner
                    sync.dma_start(
                        out=tabf_t[:], in_=tabf_d[:]).then_inc(s_tab, 16)
                elif i == 1:
                    sync.dma_start(
                        out=tabi_t[:], in_=tabi_d[:]).then_inc(s_tab2, 16)

        @block.scalar
        def _(scalar: bass.BassEngine):
            for i in range(N_TILES):
                j = i % OBUF
                r0 = i * P
                scalar.wait_ge(s_comp, 2 * i + 1)
                scalar.dma_start(
                    out=out_d[0, r0:r0 + P, :], in_=out_t[j][:, 0:ROW]
                ).then_inc(s_store[j], 16)
                scalar.wait_ge(s_comp, 2 * i + 2)
                scalar.dma_start(
                    out=out_d[1, r0:r0 + P, :], in_=out_t[j][:, ROW:2 * ROW]
                ).then_inc(s_store[j], 16)

        @block.vector
        def _(vector: bass.BassEngine):
            n = 0  # statically tracked s_dve count

            vector.wait_ge(s_tab, 16)   # dec table (loaded first)
            for i in range(N_TILES):
                j = i % NBUF
                jo = i % OBUF
                pr = i % 2
                if i >= 2:
                    # tile i-2 (same parity) fully retired, incl. its encode,
                    # before its temporaries are reused
                    vector.wait_ge(s_comp, 2 * (i - 1))
                vector.wait_ge(s_load[j], 32 * (i // NBUF + 1))
                if i >= OBUF:
                    vector.wait_ge(s_store[jo], 32 * (i // OBUF))

                # decode: a_lo a_hi b_lo b_hi as f32 accumulators
                for k in range(4):
                    vector.scalar_tensor_tensor(
                        out=dump[pr][k][:],
                        in0=ab_t[j][:, 512 * k:512 * k + 512],
                        scalar=1.0,
                        in1=dec,
                        op0=mybir.AluOpType.mult,
                        op1=mybir.AluOpType.mult,
                        accum_out=t6[pr][:, k:k + 1],
                    ).then_inc(s_dve, 1)
                n += 4
                vector.wait_ge(s_dve, n)
                # int cast of the four halves
                vector.tensor_copy(iv[pr][:, 0:4], t6[pr][:, 0:4]).then_inc(
                    s_dve, 1)
                n += 1
                vector.wait_ge(s_dve, n)
                # raw s16 halves sum into v4[0:2]; xor halves into v4[2:4]
                vector.tensor_tensor(
                    out=v4[pr][:, 0:2], in0=iv[pr][:, 0:2],
                    in1=iv[pr][:, 2:4],
                    op=mybir.AluOpType.add).then_inc(s_dve, 1)
                vector.tensor_tensor(
                    out=v4[pr][:, 2:4], in0=iv[pr][:, 0:2],
                    in1=iv[pr][:, 2:4],
                    op=mybir.AluOpType.bitwise_xor).then_inc(s_dve, 1)
                n += 2
                vector.wait_ge(s_dve, n)
                # carry lo->hi in place: s_hi += (s_lo >= 2^16); the raw
                # s_lo's bit 16 is erased later by the >>0/>>8 then &255
                vector.scalar_tensor_tensor(
                    out=v4[pr][:, 1:2], in0=v4[pr][:, 0:1], scalar=65536,
                    in1=v4[pr][:, 1:2],
                    op0=mybir.AluOpType.is_ge,
                    op1=mybir.AluOpType.add).then_inc(s_dve, 1)
                n += 1
                vector.wait_ge(s_dve, n)
                if i == 0:
                    vector.wait_ge(s_tab2, 16)  # shift/enc table ready
                vector.tensor_tensor(
                    out=sh8[pr][:],
                    in0=v4[pr][:, :, None].to_broadcast((P, 4, 2)),
                    in1=shifts,
                    op=mybir.AluOpType.logical_shift_right).then_inc(s_dve, 1)
                n += 1
                vector.wait_ge(s_dve, n)
                vector.tensor_scalar(
                    out=idx8[pr][:], in0=sh8[pr][:], scalar1=255,
                    scalar2=None,
                    op0=mybir.AluOpType.bitwise_and).then_inc(s_dve, 1)
                n += 1
                vector.wait_ge(s_dve, n)
                # encode in two halves so the add-half store releases early
                vector.tensor_tensor(
                    out=out_t[jo][:, 0:ROW].rearrange(
                        "p (e k) -> p e k", k=256),
                    in0=enc[:, 0:4, :],
                    in1=idx8[pr][:, 0:4, None].to_broadcast((P, 4, 256)),
                    op=mybir.AluOpType.is_equal,
                ).then_inc(s_comp, 1)
                vector.tensor_tensor(
                    out=out_t[jo][:, ROW:2 * ROW].rearrange(
                        "p (e k) -> p e k", k=256),
                    in0=enc[:, 4:8, :],
                    in1=idx8[pr][:, 4:8, None].to_broadcast((P, 4, 256)),
                    op=mybir.AluOpType.is_equal,
                ).then_inc(s_comp, 1)

    return nc


def _make_tables():
    dec = np.concatenate([np.arange(256), np.arange(256) * 256]).astype(np.float32)
    tabf = np.tile(dec[None, :], (P, 1))
    enc = np.tile(np.arange(256, dtype=np.int64), 8)
    shifts = np.array([0, 8] * 4, np.int64)
    tabi = np.tile(np.concatenate([enc, shifts]).astype(np.int32)[None, :],
                   (P, 1))
    return tabf, tabi


_NC_CACHE = {}


def _get_nc(variant: str = "main"):
    if variant not in _NC_CACHE:
        _NC_CACHE[variant] = _build_nc()
    return _NC_CACHE[variant]


def _run(a: np.ndarray, b: np.ndarray, **spmd_kwargs):
    assert a.shape == (B, 4, 256) and b.shape == (B, 4, 256)
    a2 = np.ascontiguousarray(a, dtype=np.float32).reshape(B, ROW)
    b2 = np.ascontiguousarray(b, dtype=np.float32).reshape(B, ROW)
    tabf, tabi = _make_tables()
    in_maps = [
        {
            "a": a2[i * B_LOC:(i + 1) * B_LOC],
            "b": b2[i * B_LOC:(i + 1) * B_LOC],
            "tabf": tabf,
            "tabi": tabi,
        }
        for i in range(N_CORES)
    ]
    nc = _get_nc()
    kr = run_bass_kernel_spmd(nc, in_maps, list(range(N_CORES)), **spmd_kwargs)
    shards = [kr.results[i]["out"] for i in range(N_CORES)]
    out = np.concatenate(shards, axis=1).reshape(2, B, 4, 256)
    return out, kr


def kernel(a: np.ndarray, b: np.ndarray) -> np.ndarray:
    out, _ = _run(a, b)
    return out


# revision 13
# speedup vs baseline: 1.3301x; 1.1676x over previous
"""MoE-ALU (add with carry + xor over one-hot byte encodings) on 8 NeuronCores.

Semantics (validated against the jax reference bit-exactly): inputs a, b are
exact one-hot byte encodings [B, 4, 256] (little-endian bytes of 32-bit ints);
with SCALE=100 every softmax in the reference collapses to an exact one-hot, so

    out[0] = one_hot bytes of (a_int + b_int) mod 2^32
    out[1] = one_hot bytes of (a_int ^ b_int)

Device kernel (pure data parallel, batch sharded over 8 cores), raw Bass
(this toolchain's walrus encodes at most ONE sync wait per instruction, so
Tile-generated schedules don't compile; manual sems with standalone waits do).

HBM roofline: 32 MB in + 32 MB out per core at ~358 GB/s -> ~190 us floor.
The DVE is the secondary constraint: its serialized op stream (measured: ops
never overlap; each op that follows a semaphore wait pays ~130 ns dispatch
latency) must stay under the DMA rate of ~5.9 us per 128-row tile.

v4 structure -- two-tile pairs, software-pipelined:

  decode  per tile: 4x scalar_tensor_tensor with accum_out (multiply one
          512-col segment of the [128, 2048] a|b tile by the
          [0..255 | 0,256,...,65280] pattern, reduce in one op)
          -> a_lo a_hi b_lo b_hi (16-bit halves, f32-exact)
  ints    per PAIR of tiles (strided APs halve the op count): one f32->i32
          cast [128,8], add -> [s_lo s_hi]x2, xor -> [x_lo x_hi]x2, carry
          folded in place into the high halves only ((s_lo>=2^16)+s_hi; the
          raw s_lo's bit 16 never survives the later >>0/>>8 then &255),
          shift/mask -> 16 byte indices
  encode  per tile: two is_equal [128, 4, 256] of the int iota table against
          stride-0-broadcast indices, writing f32 one-hots directly

  The emission order interleaves pair p's eight decode STTs between the
  dependent ops of pair p-1's int chain, so every RAW semaphore wait is
  already satisfied when the sequencer reaches it (predecessor retired >=1
  long op earlier) and the DVE never idles on sem latency.

  engines: SyncE issues input DMAs (tile-0 data before the tables; ab slots
  released as soon as the decode STTs that read them retire, via static
  s_dve thresholds), ScalarE issues output DMAs, VectorE computes. ACT and
  GpSimd do no streaming work on purpose: measured on this part, a
  concurrent ACT stream slows every DVE op ~20%.

  DVE ops do NOT self-interlock (measured: removing sync gives stale reads),
  so every same-engine RAW step still waits on the monotonically counted DVE
  semaphore; per-pair temporaries are parity-double-buffered.
"""
from contextlib import ExitStack

import numpy as np

import concourse.bass as bass
from concourse import mybir
from concourse.bass_utils import run_bass_kernel_spmd

F32 = mybir.dt.float32
I32 = mybir.dt.int32

P = 128
N_CORES = 8
B = 32768
B_LOC = B // N_CORES          # 4096 rows per core
ROW = 4 * 256                 # 1024 f32 per row per tensor
N_TILES = B_LOC // P          # 32
N_PAIRS = N_TILES // 2        # 16

NBUF = 9                      # input buffer slots (one tile each)
OBUF = 8                      # output buffer slots

TABI_COLS = 2048 + 16         # encode iota x8 | shift pattern x2 tiles


def _schedule():
    """Emission order for the vector engine (single source of truth for the
    s_dve counter, shared by the sync/vector closures)."""
    ev = []
    ev += [("stt", 0, m) for m in range(8)]
    for p in range(1, N_PAIRS):
        q = p - 1
        ev += [
            ("stt", p, 0), ("cast", q),
            ("stt", p, 1), ("add", q),
            ("stt", p, 2), ("xor", q),
            ("stt", p, 3), ("isge", q),
            ("stt", p, 4), ("shift", q),
            ("stt", p, 5), ("and", q),
            ("stt", p, 6), ("iseq", 2 * q, 0),
            ("stt", p, 7), ("iseq", 2 * q, 1),
            ("iseq", 2 * q + 1, 0), ("iseq", 2 * q + 1, 1),
        ]
    q = N_PAIRS - 1
    ev += [("cast", q), ("add", q), ("xor", q), ("isge", q), ("shift", q),
           ("and", q), ("iseq", 2 * q, 0), ("iseq", 2 * q, 1),
           ("iseq", 2 * q + 1, 0), ("iseq", 2 * q + 1, 1)]

    after = {}
    n = 0
    for e in ev:
        if e[0] != "iseq":          # iseq increments s_comp, not s_dve
            n += 1
            after[e] = n
    return ev, after


EVENTS, AFTER = _schedule()
# ab slot of tile t is free once the last decode STT reading it retires
RELEASE = {}
for _p in range(N_PAIRS):
    RELEASE[2 * _p] = AFTER[("stt", _p, 3)]
    RELEASE[2 * _p + 1] = AFTER[("stt", _p, 7)]


def _build_nc() -> bass.Bass:
    nc = bass.Bass(trn_type="TRN2")
    a_d = nc.dram_tensor("a", [B_LOC, ROW], F32, kind="ExternalInput")
    b_d = nc.dram_tensor("b", [B_LOC, ROW], F32, kind="ExternalInput")
    tabf_d = nc.dram_tensor("tabf", [P, 512], F32, kind="ExternalInput")
    tabi_d = nc.dram_tensor("tabi", [P, TABI_COLS], I32, kind="ExternalInput")
    out_d = nc.dram_tensor("out", [2, B_LOC, ROW], F32, kind="ExternalOutput")

    with ExitStack() as ctx:
        sb = lambda name, shape, dt: ctx.enter_context(
            nc.sbuf_tensor(name, shape, dt))
        tabf_t = sb("tabf_t", [P, 512], F32)
        tabi_t = sb("tabi_t", [P, TABI_COLS], I32)
        ab_t = [sb(f"ab_t{k}", [P, 2 * ROW], F32) for k in range(NBUF)]
        out_t = [sb(f"out_t{k}", [P, 2 * ROW], F32) for k in range(OBUF)]
        dump = [[sb(f"dump{c}_{k}", [P, 512], F32) for k in range(4)]
                for c in range(2)]
        # parity-double-buffered per-pair temporaries
        t8 = [sb(f"t8_{c}", [P, 8], F32) for c in range(2)]
        iv8 = [sb(f"iv8_{c}", [P, 8], I32) for c in range(2)]
        v8 = [sb(f"v8_{c}", [P, 8], I32) for c in range(2)]
        sh16 = [sb(f"sh16_{c}", [P, 16], I32) for c in range(2)]
        idx16 = [sb(f"idx16_{c}", [P, 16], I32) for c in range(2)]

        dec = tabf_t[:]
        enc = tabi_t[:, 0:2048].rearrange("p (e k) -> p e k", k=256)
        shifts = tabi_t[:, 2048:2064].rearrange("p (a two) -> p a two", two=2)

        s_tab = ctx.enter_context(nc.semaphore("s_tab"))
        s_tab2 = ctx.enter_context(nc.semaphore("s_tab2"))
        s_load = [ctx.enter_context(nc.semaphore(f"s_load{j}"))
                  for j in range(NBUF)]
        s_store = [ctx.enter_context(nc.semaphore(f"s_store{j}"))
                   for j in range(OBUF)]
        s_comp = ctx.enter_context(nc.semaphore("s_comp"))
        s_dve = ctx.enter_context(nc.semaphore("s_dve"))

        block = ctx.enter_context(nc.Block())

        @block.sync
        def _(sync: bass.BassEngine):
            for i in range(N_TILES):
                j = i % NBUF
                if i >= NBUF:
                    sync.wait_ge(s_dve, RELEASE[i - NBUF])
                r0 = i * P
                sync.dma_start(
                    out=ab_t[j][:, 0:ROW], in_=a_d[r0:r0 + P, :]
                ).then_inc(s_load[j], 16)
                sync.dma_start(
                    out=ab_t[j][:, ROW:2 * ROW], in_=b_d[r0:r0 + P, :]
                ).then_inc(s_load[j], 16)
                if i == 0:
                    # tables after tile-0 data so compute starts sooner
                    sync.dma_start(
                        out=tabf_t[:], in_=tabf_d[:]).then_inc(s_tab, 16)
                elif i == 1:
                    sync.dma_start(
                        out=tabi_t[:], in_=tabi_d[:]).then_inc(s_tab2, 16)

        @block.scalar
        def _(scalar: bass.BassEngine):
            for i in range(N_TILES):
                j = i % OBUF
                r0 = i * P
                scalar.wait_ge(s_comp, 2 * i + 1)
                scalar.dma_start(
                    out=out_d[0, r0:r0 + P, :], in_=out_t[j][:, 0:ROW]
                ).then_inc(s_store[j], 16)
                scalar.wait_ge(s_comp, 2 * i + 2)
                scalar.dma_start(
                    out=out_d[1, r0:r0 + P, :], in_=out_t[j][:, ROW:2 * ROW]
                ).then_inc(s_store[j], 16)

        @block.vector
        def _(vector: bass.BassEngine):
            # two-tile views: [p, tile, field]
            ivv = [iv8[c][:].rearrange("p (t f) -> p t f", f=4)
                   for c in range(2)]
            vvv = [v8[c][:].rearrange("p (t f) -> p t f", f=4)
                   for c in range(2)]

            vector.wait_ge(s_tab, 16)   # dec table (loaded first)
            for e in EVENTS:
                kind = e[0]
                if kind == "stt":
                    _, p, m = e
                    c = p % 2
                    t = 2 * p + (m // 4)        # tile this STT reads
                    j = t % NBUF
                    if m == 0 and p >= 2:
                        # t8 parity reuse: pair p-2's cast has consumed it
                        vector.wait_ge(s_dve, AFTER[("cast", p - 2)])
                    if m % 4 == 0:
                        vector.wait_ge(s_load[j], 32 * (t // NBUF + 1))
                    vector.scalar_tensor_tensor(
                        out=dump[c][m % 4][:],
                        in0=ab_t[j][:, 512 * (m % 4):512 * (m % 4) + 512],
                        scalar=1.0,
                        in1=dec,
                        op0=mybir.AluOpType.mult,
                        op1=mybir.AluOpType.mult,
                        accum_out=t8[c][:, m:m + 1],
                    ).then_inc(s_dve, 1)
                elif kind == "cast":
                    _, q = e
                    c = q % 2
                    vector.wait_ge(s_dve, AFTER[("stt", q, 7)])
                    vector.tensor_copy(iv8[c][:], t8[c][:]).then_inc(s_dve, 1)
                elif kind == "add":
                    _, q = e
                    c = q % 2
                    vector.wait_ge(s_dve, AFTER[("cast", q)])
                    vector.tensor_tensor(
                        out=vvv[c][:, :, 0:2], in0=ivv[c][:, :, 0:2],
                        in1=ivv[c][:, :, 2:4],
                        op=mybir.AluOpType.add).then_inc(s_dve, 1)
                elif kind == "xor":
                    _, q = e
                    c = q % 2
                    vector.tensor_tensor(
                        out=vvv[c][:, :, 2:4], in0=ivv[c][:, :, 0:2],
                        in1=ivv[c][:, :, 2:4],
                        op=mybir.AluOpType.bitwise_xor).then_inc(s_dve, 1)
                elif kind == "isge":
                    _, q = e
                    c = q % 2
                    vector.wait_ge(s_dve, AFTER[("add", q)])
                    # carry lo->hi in place: s_hi += (s_lo >= 2^16)
                    vector.scalar_tensor_tensor(
                        out=vvv[c][:, :, 1:2], in0=vvv[c][:, :, 0:1],
                        scalar=65536,
                        in1=vvv[c][:, :, 1:2],
                        op0=mybir.AluOpType.is_ge,
                        op1=mybir.AluOpType.add).then_inc(s_dve, 1)
                elif kind == "shift":
                    _, q = e
                    c = q % 2
                    if q == 0:
                        vector.wait_ge(s_tab2, 16)  # shift/enc table ready
                    vector.wait_ge(s_dve, AFTER[("isge", q)])
                    vector.tensor_tensor(
                        out=sh16[c][:],
                        in0=v8[c][:, :, None].to_broadcast((P, 8, 2)),
                        in1=shifts,
                        op=mybir.AluOpType.logical_shift_right,
                    ).then_inc(s_dve, 1)
                elif kind == "and":
                    _, q = e
                    c = q % 2
                    if q >= 2:
                        # idx16 parity reuse: pair q-2's encodes retired
                        vector.wait_ge(s_comp, 4 * (q - 1))
                    vector.wait_ge(s_dve, AFTER[("shift", q)])
                    vector.tensor_scalar(
                        out=idx16[c][:], in0=sh16[c][:], scalar1=255,
                        scalar2=None,
                        op0=mybir.AluOpType.bitwise_and).then_inc(s_dve, 1)
                else:  # iseq
                    _, t, h = e
                    c = (t // 2) % 2
                    jo = t % OBUF
                    off = 8 * (t % 2) + 4 * h
                    if h == 0:
                        vector.wait_ge(s_dve, AFTER[("and", t // 2)])
                        if t >= OBUF:
                            vector.wait_ge(s_store[jo], 32 * (t // OBUF))
                    vector.tensor_tensor(
                        out=out_t[jo][:, ROW * h:ROW * (h + 1)].rearrange(
                            "p (e k) -> p e k", k=256),
                        in0=enc[:, 4 * h:4 * h + 4, :],
                        in1=idx16[c][:, off:off + 4, None].to_broadcast(
                            (P, 4, 256)),
                        op=mybir.AluOpType.is_equal,
                    ).then_inc(s_comp, 1)

    return nc


def _make_tables():
    dec = np.concatenate([np.arange(256), np.arange(256) * 256]).astype(np.float32)
    tabf = np.tile(dec[None, :], (P, 1))
    enc = np.tile(np.arange(256, dtype=np.int64), 8)
    shifts = np.array([0, 8] * 8, np.int64)
    tabi = np.tile(np.concatenate([enc, shifts]).astype(np.int32)[None, :],
                   (P, 1))
    return tabf, tabi


_NC_CACHE = {}


def _get_nc(variant: str = "main"):
    if variant not in _NC_CACHE:
        _NC_CACHE[variant] = _build_nc()
    return _NC_CACHE[variant]


def _run(a: np.ndarray, b: np.ndarray, **spmd_kwargs):
    assert a.shape == (B, 4, 256) and b.shape == (B, 4, 256)
    a2 = np.ascontiguousarray(a, dtype=np.float32).reshape(B, ROW)
    b2 = np.ascontiguousarray(b, dtype=np.float32).reshape(B, ROW)
    tabf, tabi = _make_tables()
    in_maps = [
        {
            "a": a2[i * B_LOC:(i + 1) * B_LOC],
            "b": b2[i * B_LOC:(i + 1) * B_LOC],
            "tabf": tabf,
            "tabi": tabi,
        }
        for i in range(N_CORES)
    ]
    nc = _get_nc()
    kr = run_bass_kernel_spmd(nc, in_maps, list(range(N_CORES)), **spmd_kwargs)
    shards = [kr.results[i]["out"] for i in range(N_CORES)]
    out = np.concatenate(shards, axis=1).reshape(2, B, 4, 256)
    return out, kr


def kernel(a: np.ndarray, b: np.ndarray) -> np.ndarray:
    out, _ = _run(a, b)
    return out


# revision 16
# speedup vs baseline: 1.4744x; 1.1084x over previous
"""MoE-ALU (add with carry + xor over one-hot byte encodings) on 8 NeuronCores.

Semantics (validated against the jax reference bit-exactly): inputs a, b are
exact one-hot byte encodings [B, 4, 256] (little-endian bytes of 32-bit ints);
with SCALE=100 every softmax in the reference collapses to an exact one-hot, so

    out[0] = one_hot bytes of (a_int + b_int) mod 2^32
    out[1] = one_hot bytes of (a_int ^ b_int)

Device kernel (pure data parallel, batch sharded over 8 cores), raw Bass
(this toolchain's walrus encodes at most ONE sync wait per instruction, so
Tile-generated schedules don't compile; manual sems with standalone waits do).

The kernel is HBM-bound. The one-hot inputs are exactly representable in
bf16, so the host-side shard prep casts and interleaves a|b into one bf16
tensor: input traffic halves to 16 MB/core; output must stay f32 (32 MB).
Floor: ~48 MB/core at ~345 GB/s effective -> ~140 us of DMA wire time.

v5 structure -- two-tile pairs, software-pipelined DVE, bf16 input path:

  load    one 1 MB DMA per tile pair (both tiles' a|b rows, bf16)
  decode  per tile: 4x scalar_tensor_tensor with accum_out (multiply one
          512-col bf16 segment by the [0..255 | 0,256,...,65280] bf16
          pattern, reduce in one op; both streams 16-bit so the DVE runs
          packed) -> a_lo a_hi b_lo b_hi (16-bit halves, f32-exact)
  ints    per PAIR of tiles (strided APs halve the op count): one f32->i32
          cast [128,8], add -> [s_lo s_hi]x2, xor -> [x_lo x_hi]x2, carry
          folded in place into the high halves only ((s_lo>=2^16)+s_hi; the
          raw s_lo's bit 16 never survives the later >>0/>>8 then &255),
          shift/mask -> 16 byte indices
  encode  per tile: two is_equal [128, 4, 256] of the int iota table against
          stride-0-broadcast indices, writing f32 one-hots directly
  store   one 1 MB DMA per tile (both output planes via a strided DRAM AP)

  The emission order interleaves pair p's eight decode STTs between the
  dependent ops of pair p-1's int chain, so every RAW semaphore wait is
  already satisfied when the sequencer reaches it and the DVE never idles
  on sem latency (~130 ns per unsatisfied wait otherwise).

  engines: SyncE issues input DMAs (pair-0 data before the tables; pair
  slots released as soon as the decode STTs that read them retire, via
  static s_dve thresholds), ScalarE issues output DMAs, VectorE computes.
  ACT and GpSimd do no streaming work on purpose: measured on this part, a
  concurrent ACT stream slows every DVE op ~20%.

  DVE ops do NOT self-interlock (measured: removing sync gives stale reads),
  so every same-engine RAW step still waits on the monotonically counted DVE
  semaphore; per-pair temporaries are parity-double-buffered.
"""
from contextlib import ExitStack

import ml_dtypes
import numpy as np

import concourse.bass as bass
from concourse import mybir
from concourse.bass_utils import run_bass_kernel_spmd

F32 = mybir.dt.float32
I32 = mybir.dt.int32
BF16 = mybir.dt.bfloat16
NP_BF16 = ml_dtypes.bfloat16

P = 128
N_CORES = 8
B = 32768
B_LOC = B // N_CORES          # 4096 rows per core
ROW = 4 * 256                 # 1024 elements per row per tensor
N_TILES = B_LOC // P          # 32
N_PAIRS = N_TILES // 2        # 16

NPBUF = 6                     # input buffer slots (one tile PAIR each)
OBUF = 8                      # output buffer slots

TABI_COLS = 2048 + 16         # encode iota x8 | shift pattern x2 tiles


def _schedule():
    """Emission order for the vector engine (single source of truth for the
    s_dve counter, shared by the sync/vector closures)."""
    ev = []
    ev += [("stt", 0, m) for m in range(8)]
    for p in range(1, N_PAIRS):
        q = p - 1
        ev += [
            ("stt", p, 0), ("cast", q),
            ("stt", p, 1), ("add", q),
            ("stt", p, 2), ("xor", q),
            ("stt", p, 3), ("isge", q),
            ("stt", p, 4), ("shift", q),
            ("stt", p, 5), ("and", q),
            ("stt", p, 6), ("iseq", 2 * q, 0),
            ("stt", p, 7), ("iseq", 2 * q, 1),
            ("iseq", 2 * q + 1, 0), ("iseq", 2 * q + 1, 1),
        ]
    q = N_PAIRS - 1
    ev += [("cast", q), ("add", q), ("xor", q), ("isge", q), ("shift", q),
           ("and", q), ("iseq", 2 * q, 0), ("iseq", 2 * q, 1),
           ("iseq", 2 * q + 1, 0), ("iseq", 2 * q + 1, 1)]

    after = {}
    n = 0
    for e in ev:
        if e[0] != "iseq":          # iseq increments s_comp, not s_dve
            n += 1
            after[e] = n
    return ev, after


EVENTS, AFTER = _schedule()
# input pair slot of pair p is free once its last decode STT retires
RELEASE_PAIR = {p: AFTER[("stt", p, 7)] for p in range(N_PAIRS)}


def _build_nc() -> bass.Bass:
    nc = bass.Bass(trn_type="TRN2")
    ab_d = nc.dram_tensor("ab", [B_LOC, 2 * ROW], BF16, kind="ExternalInput")
    tabf_d = nc.dram_tensor("tabf", [P, 512], BF16, kind="ExternalInput")
    tabi_d = nc.dram_tensor("tabi", [P, TABI_COLS], I32, kind="ExternalInput")
    out_d = nc.dram_tensor("out", [2, B_LOC, ROW], F32, kind="ExternalOutput")

    with ExitStack() as ctx:
        sb = lambda name, shape, dt: ctx.enter_context(
            nc.sbuf_tensor(name, shape, dt))
        tabf_t = sb("tabf_t", [P, 512], BF16)
        tabi_t = sb("tabi_t", [P, TABI_COLS], I32)
        abp_t = [sb(f"abp_t{k}", [P, 2 * 2 * ROW], BF16) for k in range(NPBUF)]
        out_t = [sb(f"out_t{k}", [P, 2 * ROW], F32) for k in range(OBUF)]
        dump = [[sb(f"dump{c}_{k}", [P, 512], BF16) for k in range(8)]
                for c in range(2)]
        # parity-double-buffered per-pair temporaries
        t8 = [sb(f"t8_{c}", [P, 8], F32) for c in range(2)]
        iv8 = [sb(f"iv8_{c}", [P, 8], I32) for c in range(2)]
        v8 = [sb(f"v8_{c}", [P, 8], I32) for c in range(2)]
        sh16 = [sb(f"sh16_{c}", [P, 16], I32) for c in range(2)]
        idx16 = [sb(f"idx16_{c}", [P, 16], I32) for c in range(2)]

        dec = tabf_t[:]
        enc = tabi_t[:, 0:2048].rearrange("p (e k) -> p e k", k=256)
        shifts = tabi_t[:, 2048:2064].rearrange("p (a two) -> p a two", two=2)

        s_tab = ctx.enter_context(nc.semaphore("s_tab"))
        s_tab2 = ctx.enter_context(nc.semaphore("s_tab2"))
        s_load = [ctx.enter_context(nc.semaphore(f"s_load{j}"))
                  for j in range(NPBUF)]
        s_store = [ctx.enter_context(nc.semaphore(f"s_store{j}"))
                   for j in range(OBUF)]
        s_comp = ctx.enter_context(nc.semaphore("s_comp"))
        s_dve = ctx.enter_context(nc.semaphore("s_dve"))

        block = ctx.enter_context(nc.Block())

        @block.sync
        def _(sync: bass.BassEngine):
            for p in range(N_PAIRS):
                jp = p % NPBUF
                if p >= NPBUF:
                    sync.wait_ge(s_dve, RELEASE_PAIR[p - NPBUF])
                r0 = 2 * P * p
                sync.dma_start(
                    out=abp_t[jp][:].rearrange("p (two c) -> p two c", two=2),
                    in_=ab_d[r0:r0 + 2 * P, :].rearrange(
                        "(two p) c -> p two c", two=2),
                ).then_inc(s_load[jp], 16)
                if p == 0:
                    # tables after pair-0 data so compute starts sooner
                    sync.dma_start(
                        out=tabf_t[:], in_=tabf_d[:]).then_inc(s_tab, 16)
                elif p == 1:
                    sync.dma_start(
                        out=tabi_t[:], in_=tabi_d[:]).then_inc(s_tab2, 16)

        @block.scalar
        def _(scalar: bass.BassEngine):
            for i in range(N_TILES):
                j = i % OBUF
                r0 = i * P
                scalar.wait_ge(s_comp, 2 * i + 2)
                scalar.dma_start(
                    out=out_d[:, r0:r0 + P, :].rearrange(
                        "two p c -> p two c"),
                    in_=out_t[j][:].rearrange("p (two c) -> p two c", two=2),
                ).then_inc(s_store[j], 16)

        @block.vector
        def _(vector: bass.BassEngine):
            # two-tile views: [p, tile, field]
            ivv = [iv8[c][:].rearrange("p (t f) -> p t f", f=4)
                   for c in range(2)]
            vvv = [v8[c][:].rearrange("p (t f) -> p t f", f=4)
                   for c in range(2)]

            vector.wait_ge(s_tab, 16)   # dec table (loaded first)
            for e in EVENTS:
                kind = e[0]
                if kind == "stt":
                    _, p, m = e
                    c = p % 2
                    jp = p % NPBUF
                    if m == 0:
                        if p >= 2:
                            # t8 parity reuse: pair p-2's cast consumed it
                            vector.wait_ge(s_dve, AFTER[("cast", p - 2)])
                        vector.wait_ge(s_load[jp], 16 * (p // NPBUF + 1))
                    off = 512 * m          # tile (m//4), segment (m%4)
                    vector.scalar_tensor_tensor(
                        out=dump[c][m][:],
                        in0=abp_t[jp][:, off:off + 512],
                        scalar=1.0,
                        in1=dec,
                        op0=mybir.AluOpType.mult,
                        op1=mybir.AluOpType.mult,
                        accum_out=t8[c][:, m:m + 1],
                    ).then_inc(s_dve, 1)
                elif kind == "cast":
                    _, q = e
                    c = q % 2
                    vector.wait_ge(s_dve, AFTER[("stt", q, 7)])
                    vector.tensor_copy(iv8[c][:], t8[c][:]).then_inc(s_dve, 1)
                elif kind == "add":
                    _, q = e
                    c = q % 2
                    vector.wait_ge(s_dve, AFTER[("cast", q)])
                    vector.tensor_tensor(
                        out=vvv[c][:, :, 0:2], in0=ivv[c][:, :, 0:2],
                        in1=ivv[c][:, :, 2:4],
                        op=mybir.AluOpType.add).then_inc(s_dve, 1)
                elif kind == "xor":
                    _, q = e
                    c = q % 2
                    vector.tensor_tensor(
                        out=vvv[c][:, :, 2:4], in0=ivv[c][:, :, 0:2],
                        in1=ivv[c][:, :, 2:4],
                        op=mybir.AluOpType.bitwise_xor).then_inc(s_dve, 1)
                elif kind == "isge":
                    _, q = e
                    c = q % 2
                    vector.wait_ge(s_dve, AFTER[("add", q)])
                    # carry lo->hi in place: s_hi += (s_lo >= 2^16)
                    vector.scalar_tensor_tensor(
                        out=vvv[c][:, :, 1:2], in0=vvv[c][:, :, 0:1],
                        scalar=65536,
                        in1=vvv[c][:, :, 1:2],
                        op0=mybir.AluOpType.is_ge,
                        op1=mybir.AluOpType.add).then_inc(s_dve, 1)
                elif kind == "shift":
                    _, q = e
                    c = q % 2
                    if q == 0:
                        vector.wait_ge(s_tab2, 16)  # shift/enc table ready
                    vector.wait_ge(s_dve, AFTER[("isge", q)])
                    vector.tensor_tensor(
                        out=sh16[c][:],
                        in0=v8[c][:, :, None].to_broadcast((P, 8, 2)),
                        in1=shifts,
                        op=mybir.AluOpType.logical_shift_right,
                    ).then_inc(s_dve, 1)
                elif kind == "and":
                    _, q = e
                    c = q % 2
                    if q >= 2:
                        # idx16 parity reuse: pair q-2's encodes retired
                        vector.wait_ge(s_comp, 4 * (q - 1))
                    vector.wait_ge(s_dve, AFTER[("shift", q)])
                    vector.tensor_scalar(
                        out=idx16[c][:], in0=sh16[c][:], scalar1=255,
                        scalar2=None,
                        op0=mybir.AluOpType.bitwise_and).then_inc(s_dve, 1)
                else:  # iseq
                    _, t, h = e
                    c = (t // 2) % 2
                    jo = t % OBUF
                    off = 8 * (t % 2) + 4 * h
                    if h == 0:
                        vector.wait_ge(s_dve, AFTER[("and", t // 2)])
                        if t >= OBUF:
                            vector.wait_ge(s_store[jo], 16 * (t // OBUF))
                    vector.tensor_tensor(
                        out=out_t[jo][:, ROW * h:ROW * (h + 1)].rearrange(
                            "p (e k) -> p e k", k=256),
                        in0=enc[:, 4 * h:4 * h + 4, :],
                        in1=idx16[c][:, off:off + 4, None].to_broadcast(
                            (P, 4, 256)),
                        op=mybir.AluOpType.is_equal,
                    ).then_inc(s_comp, 1)

    return nc


def _make_tables():
    dec = np.concatenate([np.arange(256), np.arange(256) * 256])
    tabf = np.tile(dec[None, :], (P, 1)).astype(NP_BF16)
    enc = np.tile(np.arange(256, dtype=np.int64), 8)
    shifts = np.array([0, 8] * 8, np.int64)
    tabi = np.tile(np.concatenate([enc, shifts]).astype(np.int32)[None, :],
                   (P, 1))
    return tabf, tabi


_NC_CACHE = {}


def _get_nc(variant: str = "main"):
    if variant not in _NC_CACHE:
        _NC_CACHE[variant] = _build_nc()
    return _NC_CACHE[variant]


def _pack_inputs(a: np.ndarray, b: np.ndarray) -> np.ndarray:
    """Interleave a|b rows and cast to bf16 (one-hot 0/1 is exact)."""
    ab = np.empty((B, 2 * ROW), NP_BF16)
    ab[:, 0:ROW] = a.reshape(B, ROW)
    ab[:, ROW:2 * ROW] = b.reshape(B, ROW)
    return ab


def _run(a: np.ndarray, b: np.ndarray, **spmd_kwargs):
    assert a.shape == (B, 4, 256) and b.shape == (B, 4, 256)
    ab = _pack_inputs(np.ascontiguousarray(a, dtype=np.float32),
                      np.ascontiguousarray(b, dtype=np.float32))
    tabf, tabi = _make_tables()
    in_maps = [
        {
            "ab": ab[i * B_LOC:(i + 1) * B_LOC],
            "tabf": tabf,
            "tabi": tabi,
        }
        for i in range(N_CORES)
    ]
    nc = _get_nc()
    kr = run_bass_kernel_spmd(nc, in_maps, list(range(N_CORES)), **spmd_kwargs)
    shards = [kr.results[i]["out"] for i in range(N_CORES)]
    out = np.concatenate(shards, axis=1).reshape(2, B, 4, 256)
    return out, kr


def kernel(a: np.ndarray, b: np.ndarray) -> np.ndarray:
    out, _ = _run(a, b)
    return out
